# revision 1
# baseline (speedup 1.0000x reference)
"""AttnBlock (GroupNorm + 1x1-conv QKV self-attention + residual) on 8 trn2 cores.

Sharding: data-parallel over batch (16 batches -> 2 per core), weights replicated.
All heavy matmuls run in bf16 (wo has gain 1e-5, so attention-path rounding is
attenuated ~1e-5 in the final output; the fp32 residual path x + ... is exact).

Layout plan (per batch element, per core):
  x, h        [c, n]   c on partitions (4 tiles of 128), n=1024 free
  q, k        [o, n]   via matmul(lhsT=wT[c,o], rhs=h[c,n])
  vT          [m, c]   via matmul(lhsT=h[c,m], rhs=wvT[c,o])  (no transposes!)
  scores s    [m, n]   via matmul(lhsT=k[o,m], rhs=q[o,n])
  p=exp(s*sc) [m, n]   ACT, psum->sbuf bf16
  denom       [128,n]  DVE reduce over the m-tile dim of p (two halves, each
                       issued as soon as its 4 m-tiles exist), then one
                       ones-matmul for cross-partition sum + broadcast
  hv          [c, n]   matmul(lhsT=vT[m,c], rhs=p[m,n]) * (1/denom)
  out         [o, n]   matmul(lhsT=woT[c,o], rhs=hv[c,n]) + bo_eff + x  -> DRAM
  (bv is folded into bo_eff = bo + wo@bv on the host: softmax rows sum to 1)

Schedule notes: DMA order is consts -> x(batch0) -> weights -> x(batch1) so
GroupNorm starts immediately; GN stats for all 4 channel tiles are fused into
one PE reduce + one PE broadcast; PSUM evictions run on ACT (Identity/Copy)
to keep the DVE off the critical path; the softmax denominator reduce runs
on the DVE (off the PE) with its first half hidden under the scores phase;
a PE warmup burst bridges the DMA+GroupNorm head so matmuls start warm.
"""

from contextlib import ExitStack

import numpy as np
import ml_dtypes

import concourse.bass as bass
from concourse import bacc
import concourse.mybir as mybir
import concourse.tile as tile
from concourse.bass import ts
from concourse.bass_utils import run_bass_kernel_spmd

B, C, H, W = 16, 512, 32, 32
N = H * W            # 1024 spatial positions
NCORES = 8
BPC = B // NCORES    # batches per core
GROUPS = 32
CPG = C // GROUPS    # 16 channels per group
CT = C // 128        # 4 channel tiles
GPT = 128 // CPG     # 8 groups per channel tile
NT = N // 128        # 8 spatial tiles
NH = N // 512        # 2 free-dim halves (psum bank = 512 fp32)
EPS = 1e-5
SCALE = 1.0 / float(np.sqrt(C))

F32 = mybir.dt.float32
BF16 = mybir.dt.bfloat16
FP8 = mybir.dt.float8e4
USE_FP8 = __import__("os").environ.get("KFP8", "0") == "1"
# exp(score - 4): keeps p within fp8e4 range; the uniform shift cancels in
# the softmax division.
EXPSHIFT = -4.0
MMDT = FP8 if USE_FP8 else BF16
KPAIR = 2 if USE_FP8 else 1
PERF = mybir.MatmulPerfMode.DoubleRow if USE_FP8 else None
AF = mybir.ActivationFunctionType
OP = mybir.AluOpType

_CACHE = {}


def build_nc(reps=1):
    nc = bacc.Bacc(trn_type="TRN2")

    x_d = nc.dram_tensor("x", [BPC, CT, 128, N], F32, kind="ExternalInput")
    w_d = {
        k: nc.dram_tensor(k, [CT, 128, C], MMDT, kind="ExternalInput")
        for k in ("wqT", "wkT", "wvT", "woT")
    }
    bvec_d = nc.dram_tensor("bvec", [128, 5 * CT], F32, kind="ExternalInput")
    gmask_d = nc.dram_tensor("gmask", [128, GPT], BF16, kind="ExternalInput")
    expand_d = nc.dram_tensor("expand", [GPT, 128], BF16, kind="ExternalInput")
    out_d = nc.dram_tensor("out", [BPC, CT, 128, N], F32, kind="ExternalOutput")

    with tile.TileContext(nc) as tc, ExitStack() as ctx:
        pool = lambda *a, **kw: ctx.enter_context(tc.tile_pool(*a, **kw))
        singles = pool(name="singles", bufs=1)
        xp = pool(name="xp", bufs=2)
        hp = pool(name="hp", bufs=2)
        qkp = pool(name="qkp", bufs=1)
        vp = pool(name="vp", bufs=1)
        pp = pool(name="pp", bufs=1)
        rp = pool(name="rp", bufs=1)
        hvp = pool(name="hvp", bufs=1)
        resp = pool(name="resp", bufs=3)
        gnp = pool(name="gnp", bufs=2)
        ps_mm = pool(name="ps_mm", bufs=5, space="PSUM")
        ps_aux = pool(name="ps_aux", bufs=3, space="PSUM")
        ps_den = ps_gn = ps_aux

        # --- batch0 x first (GroupNorm stats gate everything) ---
        x_tiles = []
        for b in range(BPC):
            x_tiles.append(xp.tile([128, CT, N], F32, tag="x", name=f"x{b}"))
        for ct in range(CT):
            nc.sync.dma_start(out=x_tiles[0][:, ct, :], in_=x_d[0, ct])
        first_x_tiles = x_tiles
        # --- tiny constants (gmask gates the first PE instruction) ---
        gmask = singles.tile([128, GPT], BF16, tag="gmask")
        nc.sync.dma_start(out=gmask, in_=gmask_d.ap())
        expand = singles.tile([GPT, 128], BF16, tag="expand")
        nc.sync.dma_start(out=expand, in_=expand_d.ap())
        bvec = singles.tile([128, 5 * CT], F32, tag="bvec")
        nc.sync.dma_start(out=bvec, in_=bvec_d.ap())
        b_sb = {
            k: bvec[:, i * CT : (i + 1) * CT]
            for i, k in enumerate(("bq", "bk", "bo", "gn_scale", "gn_bias"))
        }
        ones_bf = singles.tile([128, KPAIR, 128], MMDT, tag="ones")
        nc.vector.memset(ones_bf, 1.0)
        eps_sb = singles.tile([128, 1], F32, tag="eps")
        nc.vector.memset(eps_sb, EPS)
        shift_sb = singles.tile([128, 1], F32, tag="shift")
        nc.vector.memset(shift_sb, EXPSHIFT if USE_FP8 else 0.0)
        warm_rhs = singles.tile([128, 512], BF16, tag="warm_rhs")
        nc.vector.memset(warm_rhs, 0.0)
        warm_ps = ps_aux.tile([128, 512], F32, tag="aux", name="warm_ps")
        for i in range(32):
            nc.tensor.matmul(
                warm_ps, warm_rhs[:, :128], warm_rhs,
                start=(i == 0), stop=(i == 31),
            )
        warm_out = singles.tile([128, 1], F32, tag="warm_out")
        nc.vector.tensor_copy(warm_out, warm_ps[:, 0:1])

        # --- weights, then batch1 x ---
        w_sb = {}
        for k in ("wqT", "wkT", "wvT", "woT"):
            t = singles.tile([128, CT, C], MMDT, tag=k)
            for ct in range(CT):
                nc.sync.dma_start(out=t[:, ct, :], in_=w_d[k][ct])
            w_sb[k] = t
        for b in range(1, BPC):
            for ct in range(CT):
                nc.sync.dma_start(out=x_tiles[b][:, ct, :], in_=x_d[b, ct])

      # (reps>1 re-runs the whole body for slope timing; writes are idempotent)
        for rep in range(reps):
          if rep == 0:
            x_tiles = first_x_tiles
          else:
            x_tiles = [
                xp.tile([128, CT, N], F32, tag="x", name=f"x{rep}_{b}")
                for b in range(BPC)
            ]
            for b in range(BPC):
                for ct in range(CT):
                    nc.sync.dma_start(out=x_tiles[b][:, ct, :], in_=x_d[b, ct])

          # -- GroupNorm for every batch up front (h ready before attention) --
          h_tiles = []
          for b in range(BPC):
            x_all = x_tiles[b]
            h_all = hp.tile([128, CT, N], MMDT, tag="h", name=f"h{b}")
            h_tiles.append(h_all)

            # ------------- GroupNorm (all 4 channel tiles fused) -------------
            stats = gnp.tile([128, CT, 2, 6], F32, tag="stats")
            mv_all = gnp.tile([128, CT, 2], F32, tag="mv")
            for ct in range(CT):
                xv = x_all[:, ct, :].rearrange("p (s f) -> p s f", f=512)
                for s in range(2):
                    nc.vector.bn_stats(out=stats[:, ct, s, :], in_=xv[:, s, :])
                nc.vector.bn_aggr(out=mv_all[:, ct, :], in_=stats[:, ct, :, :])
            # mv2 = [mean_c, E[x^2]_c] per channel, bf16 for the PE reduce
            mv2 = gnp.tile([128, CT, 2], BF16, tag="mv2")
            tmp4 = gnp.tile([128, CT], F32, tag="tmp4")
            nc.vector.tensor_copy(mv2[:, :, 0], mv_all[:, :, 0])
            nc.vector.tensor_tensor(tmp4, mv_all[:, :, 0], mv_all[:, :, 0],
                                    op=OP.mult)
            nc.vector.tensor_tensor(mv2[:, :, 1], tmp4, mv_all[:, :, 1],
                                    op=OP.add)
            # group stats for all 32 groups in one matmul: [8, CT*2]
            ps_g = ps_gn.tile([GPT, CT * 2], F32, tag="aux", padded_shape=[GPT, 512])
            nc.tensor.matmul(ps_g, gmask, mv2, start=True, stop=True)
            gv = ps_g.rearrange("g (c two) -> g c two", two=2)
            g2 = gnp.tile([GPT, CT, 2], F32, tag="g2")
            nc.vector.tensor_copy(g2, gv)  # [mu, E] psum -> sbuf (1 PSUM read)
            g4 = gnp.tile([GPT, CT, 4], F32, tag="g4")
            nc.vector.tensor_tensor(g4[:, :, 0], g2[:, :, 0], g2[:, :, 0],
                                    op=OP.mult)  # mu^2
            nc.vector.tensor_tensor(g4[:, :, 1], g2[:, :, 1], g4[:, :, 0],
                                    op=OP.subtract)  # var
            nc.scalar.activation(out=g4[:, :, 2], in_=g4[:, :, 1],
                                 func=AF.Sqrt, bias=eps_sb[:GPT])
            nc.vector.reciprocal(out=g4[:, :, 3], in_=g4[:, :, 2])  # rstd
            gb = gnp.tile([GPT, CT, 2], BF16, tag="gb")
            nc.vector.tensor_copy(gb[:, :, 0], g2[:, :, 0])  # mu
            nc.vector.tensor_copy(gb[:, :, 1], g4[:, :, 3])  # rstd
            # broadcast [mu, rstd] to all 128 channel partitions
            ps_bc = ps_gn.tile([128, CT * 2], F32, tag="aux", padded_shape=[128, 512])
            nc.tensor.matmul(ps_bc, expand, gb, start=True, stop=True)
            bc = ps_bc.rearrange("p (c two) -> p c two", two=2)
            mo_m = gnp.tile([128, CT], F32, tag="mo_m")
            mo_t = gnp.tile([128, CT], F32, tag="mo_t")
            mo_o = gnp.tile([128, CT], F32, tag="mo_o")
            nc.vector.tensor_tensor(mo_m, bc[:, :, 1], b_sb["gn_scale"],
                                    op=OP.mult)
            nc.vector.tensor_tensor(mo_t, bc[:, :, 0], mo_m, op=OP.mult)
            nc.vector.tensor_tensor(mo_o, b_sb["gn_bias"], mo_t,
                                    op=OP.subtract)
            for ct in range(CT):
                nc.vector.tensor_scalar(
                    out=h_all[:, ct, :], in0=x_all[:, ct, :],
                    scalar1=mo_m[:, ct : ct + 1], scalar2=mo_o[:, ct : ct + 1],
                    op0=OP.mult, op1=OP.add,
                )

          for b in range(BPC):
              x_all = x_tiles[b]
              h_all = h_tiles[b]
              q_all = qkp.tile([128, CT, N], MMDT, tag="q")
              k_all = qkp.tile([128, CT, N], MMDT, tag="k")
              vT_all = vp.tile([128, NT, C], MMDT, tag="vT")
              p_all = pp.tile([128, NT, N], MMDT, tag="p")
              recip = rp.tile([128, N], F32, tag="recip")
              hv_all = hvp.tile([128, CT, N], MMDT, tag="hv")

              # ---------------- q, k projections [o, n] ----------------
              for name, dst, bias in (("wqT", q_all, "bq"), ("wkT", k_all, "bk")):
                  for ot in range(CT):
                      for nh in range(NH):
                          ps = ps_mm.tile([128, 512], F32, tag="mm")
                          for ct in range(0, CT, KPAIR):
                              nc.tensor.matmul(
                                  ps,
                                  w_sb[name][:, ct : ct + KPAIR, ts(ot, 128)],
                                  h_all[:, ct : ct + KPAIR, ts(nh, 512)],
                                  start=(ct == 0),
                                  stop=(ct == CT - KPAIR),
                                  perf_mode=PERF,
                              )
                          nc.scalar.activation(
                              out=dst[:, ot, ts(nh, 512)], in_=ps,
                              func=AF.Identity,
                              bias=b_sb[bias][:, ot : ot + 1],
                          )

              # ---------------- vT [m, c] ----------------
              for mt in range(NT):
                  ps = ps_mm.tile([128, 512], F32, tag="mm")
                  for ct in range(0, CT, KPAIR):
                      nc.tensor.matmul(
                          ps,
                          h_all[:, ct : ct + KPAIR, ts(mt, 128)],
                          w_sb["wvT"][:, ct : ct + KPAIR, :],
                          start=(ct == 0),
                          stop=(ct == CT - KPAIR),
                          perf_mode=PERF,
                      )
                  nc.vector.tensor_copy(vT_all[:, mt, :], ps)

              # ------------- scores + exp + (lagged) denominator -------------
              psum_part = gnp.tile([128, NH, 2, 512], BF16, tag="psum_part")

              for mt in range(NT):
                  for nh in range(NH):
                      ps = ps_mm.tile([128, 512], F32, tag="mm")
                      for ot in range(0, CT, KPAIR):
                          nc.tensor.matmul(
                              ps,
                              k_all[:, ot : ot + KPAIR, ts(mt, 128)],
                              q_all[:, ot : ot + KPAIR, ts(nh, 512)],
                              start=(ot == 0),
                              stop=(ot == CT - KPAIR),
                              perf_mode=PERF,
                          )
                      nc.scalar.activation(
                          out=p_all[:, mt, ts(nh, 512)], in_=ps, func=AF.Exp,
                          scale=SCALE, bias=shift_sb,
                      )
                  if mt in (NT // 2 - 1, NT - 1):
                      hh = 0 if mt == NT // 2 - 1 else 1
                      lo = hh * (NT // 2)
                      for nh in range(NH):
                          pv = p_all[:, lo : lo + NT // 2, ts(nh, 512)]\
                              .rearrange("p m n -> p n m")
                          with nc.allow_low_precision(
                              reason="softmax denominator partials; "
                              "common-mode per column, attenuated 1e-5"
                          ):
                              nc.vector.tensor_reduce(
                                  out=psum_part[:, nh, hh, :], in_=pv,
                                  op=OP.add, axis=mybir.AxisListType.X,
                              )
              for nh in range(NH):
                  den_ps = ps_den.tile(
                      [128, 512], F32, tag="aux", name=f"den{b}_{nh}"
                  )
                  for hh in range(2):
                      nc.tensor.matmul(
                          den_ps, ones_bf, psum_part[:, nh, hh, :],
                          start=(hh == 0), stop=(hh == 1), perf_mode=PERF,
                      )
                  nc.vector.reciprocal(out=recip[:, ts(nh, 512)], in_=den_ps)

              # ---------------- hv = (v @ p) * recip ----------------
              for ct in range(CT):
                  for nh in range(NH):
                      ps = ps_mm.tile([128, 512], F32, tag="mm")
                      for mt in range(0, NT, KPAIR):
                          nc.tensor.matmul(
                              ps,
                              vT_all[:, mt : mt + KPAIR, ts(ct, 128)],
                              p_all[:, mt : mt + KPAIR, ts(nh, 512)],
                              start=(mt == 0),
                              stop=(mt == NT - KPAIR),
                              perf_mode=PERF,
                          )
                      nc.vector.tensor_tensor(
                          hv_all[:, ct, ts(nh, 512)], ps, recip[:, ts(nh, 512)],
                          op=OP.mult,
                      )

              # ---------------- out = woT.T @ hv + bo_eff + x ----------------
              for ot in range(CT):
                  res = resp.tile([128, N], F32, tag="res")
                  for nh in range(NH):
                      ps = ps_mm.tile([128, 512], F32, tag="mm")
                      for ct in range(0, CT, KPAIR):
                          nc.tensor.matmul(
                              ps,
                              w_sb["woT"][:, ct : ct + KPAIR, ts(ot, 128)],
                              hv_all[:, ct : ct + KPAIR, ts(nh, 512)],
                              start=(ct == 0),
                              stop=(ct == CT - KPAIR),
                              perf_mode=PERF,
                          )
                      nc.vector.scalar_tensor_tensor(
                          out=res[:, ts(nh, 512)], in0=ps,
                          scalar=b_sb["bo"][:, ot : ot + 1],
                          in1=x_all[:, ot, ts(nh, 512)],
                          op0=OP.add, op1=OP.add,
                      )
                      nc.sync.dma_start(
                          out=out_d[b, ot][:, ts(nh, 512)], in_=res[:, ts(nh, 512)]
                      )

    # The axon/PJRT path serializes nc without finalizing; Bacc's compile
    # passes (wait splitting, register allocation) must run first.
    nc.finalize()
    return nc


def _prep_inputs(x, gn_scale, gn_bias, wq, bq, wk, bk, wv, bv, wo, bo):
    bf = ml_dtypes.bfloat16
    xr = np.asarray(x, np.float32).reshape(B, CT, 128, N)
    shared = {}
    wdt = ml_dtypes.float8_e4m3 if USE_FP8 else bf
    for name, w in (("wqT", wq), ("wkT", wk), ("wvT", wv), ("woT", wo)):
        shared[name] = np.ascontiguousarray(
            np.asarray(w, np.float32).T
        ).astype(wdt).reshape(CT, 128, C)
    # bv folds into bo exactly: softmax rows sum to 1, so hv = hv_u/denom + bv
    # and wo @ (hv + bv) = wo @ hv + (wo @ bv).
    bo_eff = np.asarray(bo, np.float32) + (
        np.asarray(wo, np.float32) @ np.asarray(bv, np.float32)
    )
    vecs = [bq, bk, bo_eff, gn_scale, gn_bias]
    bvec = np.stack(
        [np.asarray(v, np.float32).reshape(CT, 128) for v in vecs]
    )  # [5, CT, 128]
    shared["bvec"] = np.ascontiguousarray(bvec.transpose(2, 0, 1).reshape(128, 5 * CT))
    gmask = np.zeros((128, GPT), np.float32)
    expand = np.zeros((GPT, 128), np.float32)
    for c in range(128):
        gmask[c, c // CPG] = 1.0 / CPG
        expand[c // CPG, c] = 1.0
    shared["gmask"] = gmask.astype(bf)
    shared["expand"] = expand.astype(bf)
    return [
        {"x": np.ascontiguousarray(xr[i * BPC : (i + 1) * BPC]), **shared}
        for i in range(NCORES)
    ]


def kernel(**inputs) -> np.ndarray:
    if "nc" not in _CACHE:
        _CACHE["nc"] = build_nc()
    in_maps = _prep_inputs(**inputs)
    res = run_bass_kernel_spmd(
        _CACHE["nc"], in_maps, core_ids=list(range(NCORES))
    )
    _CACHE["last_results"] = res
    out = np.concatenate(
        [np.asarray(r["out"], np.float32).reshape(BPC, C, N) for r in res.results],
        axis=0,
    )
    return out.reshape(B, C, H, W)



# revision 3
# speedup vs baseline: 1.8816x; 1.8816x over previous
"""AttnBlock (GroupNorm + 1x1-conv QKV self-attention + residual) on 8 trn2 cores.

Sharding: data-parallel over batch (16 batches -> 2 per core), weights replicated.
All heavy matmuls run in bf16 (wo has gain 1e-5, so attention-path rounding is
attenuated ~1e-5 in the final output; the fp32 residual path x + ... is exact).

Layout plan (per batch element, per core):
  x, h        [c, n]   c on partitions (4 tiles of 128), n=1024 free
  q, k        [o, n]   via matmul(lhsT=wT[c,o], rhs=h[c,n])
  vT          [m, c]   via matmul(lhsT=h[c,m], rhs=wvT[c,o])  (no transposes!)
  scores s    [m, n]   via matmul(lhsT=k[o,m], rhs=q[o,n])
  p=exp(s*sc) [m, n]   ACT, psum->sbuf bf16
  denom       [128,n]  DVE reduce over the m-tile dim of p (two halves, each
                       issued as soon as its 4 m-tiles exist), then one
                       ones-matmul for cross-partition sum + broadcast
  hv          [c, n]   matmul(lhsT=vT[m,c], rhs=p[m,n]) * (1/denom)
  out         [o, n]   matmul(lhsT=woT[c,o], rhs=hv[c,n]) + bo_eff + x  -> DRAM
  (bv is folded into bo_eff = bo + wo@bv on the host: softmax rows sum to 1)

Schedule notes: DMA order is consts -> x(batch0) -> weights -> x(batch1) so
GroupNorm starts immediately; GN stats for all 4 channel tiles are fused into
one PE reduce + one PE broadcast; PSUM evictions run on ACT (Identity/Copy)
to keep the DVE off the critical path; the softmax denominator reduce runs
on the DVE (off the PE) with its first half hidden under the scores phase;
a PE warmup burst bridges the DMA+GroupNorm head so matmuls start warm.
"""

from contextlib import ExitStack

import numpy as np
import ml_dtypes

import concourse.bass as bass
from concourse import bacc
import concourse.mybir as mybir
import concourse.tile as tile
from concourse.bass import ts
from concourse.bass_utils import run_bass_kernel_spmd

B, C, H, W = 16, 512, 32, 32
N = H * W            # 1024 spatial positions
NCORES = 8
BPC = B // NCORES    # batches per core
GROUPS = 32
CPG = C // GROUPS    # 16 channels per group
CT = C // 128        # 4 channel tiles
GPT = 128 // CPG     # 8 groups per channel tile
NT = N // 128        # 8 spatial tiles
NH = N // 512        # 2 free-dim halves (psum bank = 512 fp32)
EPS = 1e-5
SCALE = 1.0 / float(np.sqrt(C))

F32 = mybir.dt.float32
BF16 = mybir.dt.bfloat16
FP8 = mybir.dt.float8e4
USE_FP8 = __import__("os").environ.get("KFP8", "0") == "1"
# exp(score - 4): keeps p within fp8e4 range; the uniform shift cancels in
# the softmax division.
EXPSHIFT = -4.0
MMDT = FP8 if USE_FP8 else BF16
KPAIR = 2 if USE_FP8 else 1
PERF = mybir.MatmulPerfMode.DoubleRow if USE_FP8 else None
AF = mybir.ActivationFunctionType
OP = mybir.AluOpType

_CACHE = {}


def build_nc(reps=1):
    nc = bacc.Bacc(trn_type="TRN2")

    x_d = nc.dram_tensor("x", [BPC, CT, 128, N], F32, kind="ExternalInput")
    w_d = {
        k: nc.dram_tensor(k, [CT, 128, C], MMDT, kind="ExternalInput")
        for k in ("wqT", "wkT", "wvT", "woT")
    }
    bvec_d = nc.dram_tensor("bvec", [128, 5 * CT], F32, kind="ExternalInput")
    gmask_d = nc.dram_tensor("gmask", [128, GPT], BF16, kind="ExternalInput")
    expand_d = nc.dram_tensor("expand", [GPT, 128], BF16, kind="ExternalInput")
    out_d = nc.dram_tensor("out", [BPC, CT, 128, N], F32, kind="ExternalOutput")

    with tile.TileContext(nc) as tc, ExitStack() as ctx:
        pool = lambda *a, **kw: ctx.enter_context(tc.tile_pool(*a, **kw))
        singles = pool(name="singles", bufs=1)
        xp = pool(name="xp", bufs=2)
        hp = pool(name="hp", bufs=2)
        qkp = pool(name="qkp", bufs=1)
        vp = pool(name="vp", bufs=1)
        pp = pool(name="pp", bufs=1)
        rp = pool(name="rp", bufs=1)
        hvp = pool(name="hvp", bufs=1)
        resp = pool(name="resp", bufs=3)
        gnp = pool(name="gnp", bufs=2)
        ps_mm = pool(name="ps_mm", bufs=5, space="PSUM")
        ps_aux = pool(name="ps_aux", bufs=3, space="PSUM")
        ps_den = ps_gn = ps_aux

        # --- batch0 x first (GroupNorm stats gate everything) ---
        x_tiles = []
        for b in range(BPC):
            x_tiles.append(xp.tile([128, CT, N], F32, tag="x", name=f"x{b}"))
        for ct in range(CT):
            nc.sync.dma_start(out=x_tiles[0][:, ct, :], in_=x_d[0, ct])
        first_x_tiles = x_tiles
        # --- tiny constants (gmask gates the first PE instruction) ---
        gmask = singles.tile([128, GPT], BF16, tag="gmask")
        nc.sync.dma_start(out=gmask, in_=gmask_d.ap())
        expand = singles.tile([GPT, 128], BF16, tag="expand")
        nc.sync.dma_start(out=expand, in_=expand_d.ap())
        bvec = singles.tile([128, 5 * CT], F32, tag="bvec")
        nc.sync.dma_start(out=bvec, in_=bvec_d.ap())
        b_sb = {
            k: bvec[:, i * CT : (i + 1) * CT]
            for i, k in enumerate(("bq", "bk", "bo", "gn_scale", "gn_bias"))
        }
        ones_bf = singles.tile([128, 128], BF16, tag="ones")
        nc.vector.memset(ones_bf, 1.0)
        eps_sb = singles.tile([128, 1], F32, tag="eps")
        nc.vector.memset(eps_sb, EPS)
        shift_sb = singles.tile([128, 1], F32, tag="shift")
        nc.vector.memset(shift_sb, EXPSHIFT if USE_FP8 else 0.0)
        warm_rhs = singles.tile([128, 512], BF16, tag="warm_rhs")
        nc.vector.memset(warm_rhs, 0.0)
        warm_ps = ps_aux.tile([128, 512], F32, tag="aux", name="warm_ps")
        for i in range(32):
            nc.tensor.matmul(
                warm_ps, warm_rhs[:, :128], warm_rhs,
                start=(i == 0), stop=(i == 31),
            )
        warm_out = singles.tile([128, 1], F32, tag="warm_out")
        nc.vector.tensor_copy(warm_out, warm_ps[:, 0:1])

        # --- weights, then batch1 x ---
        w_sb = {}
        for k in ("wqT", "wkT", "wvT", "woT"):
            t = singles.tile([128, CT, C], MMDT, tag=k)
            for ct in range(CT):
                nc.sync.dma_start(out=t[:, ct, :], in_=w_d[k][ct])
            w_sb[k] = t
        for b in range(1, BPC):
            for ct in range(CT):
                nc.sync.dma_start(out=x_tiles[b][:, ct, :], in_=x_d[b, ct])

      # (reps>1 re-runs the whole body for slope timing; writes are idempotent)
        for rep in range(reps):
          if rep == 0:
            x_tiles = first_x_tiles
          else:
            x_tiles = [
                xp.tile([128, CT, N], F32, tag="x", name=f"x{rep}_{b}")
                for b in range(BPC)
            ]
            for b in range(BPC):
                for ct in range(CT):
                    nc.sync.dma_start(out=x_tiles[b][:, ct, :], in_=x_d[b, ct])

          # -- GroupNorm for every batch up front (h ready before attention) --
          h_tiles = []
          for b in range(BPC):
            x_all = x_tiles[b]
            h_all = hp.tile([128, CT, N], MMDT, tag="h", name=f"h{b}")
            h_tiles.append(h_all)

            # ------------- GroupNorm (all 4 channel tiles fused) -------------
            stats = gnp.tile([128, CT, 2, 6], F32, tag="stats")
            mv_all = gnp.tile([128, CT, 2], F32, tag="mv")
            for ct in range(CT):
                xv = x_all[:, ct, :].rearrange("p (s f) -> p s f", f=512)
                for s in range(2):
                    nc.vector.bn_stats(out=stats[:, ct, s, :], in_=xv[:, s, :])
                nc.vector.bn_aggr(out=mv_all[:, ct, :], in_=stats[:, ct, :, :])
            # mv2 = [mean_c, E[x^2]_c] per channel, bf16 for the PE reduce
            mv2 = gnp.tile([128, CT, 2], BF16, tag="mv2")
            tmp4 = gnp.tile([128, CT], F32, tag="tmp4")
            nc.vector.tensor_copy(mv2[:, :, 0], mv_all[:, :, 0])
            nc.vector.tensor_tensor(tmp4, mv_all[:, :, 0], mv_all[:, :, 0],
                                    op=OP.mult)
            nc.vector.tensor_tensor(mv2[:, :, 1], tmp4, mv_all[:, :, 1],
                                    op=OP.add)
            # group stats for all 32 groups in one matmul: [8, CT*2]
            ps_g = ps_gn.tile([GPT, CT * 2], F32, tag="aux", padded_shape=[GPT, 512])
            nc.tensor.matmul(ps_g, gmask, mv2, start=True, stop=True)
            gv = ps_g.rearrange("g (c two) -> g c two", two=2)
            g2 = gnp.tile([GPT, CT, 2], F32, tag="g2")
            nc.vector.tensor_copy(g2, gv)  # [mu, E] psum -> sbuf (1 PSUM read)
            g4 = gnp.tile([GPT, CT, 4], F32, tag="g4")
            nc.vector.tensor_tensor(g4[:, :, 0], g2[:, :, 0], g2[:, :, 0],
                                    op=OP.mult)  # mu^2
            nc.vector.tensor_tensor(g4[:, :, 1], g2[:, :, 1], g4[:, :, 0],
                                    op=OP.subtract)  # var
            nc.scalar.activation(out=g4[:, :, 2], in_=g4[:, :, 1],
                                 func=AF.Sqrt, bias=eps_sb[:GPT])
            nc.vector.reciprocal(out=g4[:, :, 3], in_=g4[:, :, 2])  # rstd
            gb = gnp.tile([GPT, CT, 2], BF16, tag="gb")
            nc.vector.tensor_copy(gb[:, :, 0], g2[:, :, 0])  # mu
            nc.vector.tensor_copy(gb[:, :, 1], g4[:, :, 3])  # rstd
            # broadcast [mu, rstd] to all 128 channel partitions
            ps_bc = ps_gn.tile([128, CT * 2], F32, tag="aux", padded_shape=[128, 512])
            nc.tensor.matmul(ps_bc, expand, gb, start=True, stop=True)
            bc = ps_bc.rearrange("p (c two) -> p c two", two=2)
            mo_m = gnp.tile([128, CT], F32, tag="mo_m")
            mo_t = gnp.tile([128, CT], F32, tag="mo_t")
            mo_o = gnp.tile([128, CT], F32, tag="mo_o")
            nc.vector.tensor_tensor(mo_m, bc[:, :, 1], b_sb["gn_scale"],
                                    op=OP.mult)
            nc.vector.tensor_tensor(mo_t, bc[:, :, 0], mo_m, op=OP.mult)
            nc.vector.tensor_tensor(mo_o, b_sb["gn_bias"], mo_t,
                                    op=OP.subtract)
            for ct in range(CT):
                nc.vector.tensor_scalar(
                    out=h_all[:, ct, :], in0=x_all[:, ct, :],
                    scalar1=mo_m[:, ct : ct + 1], scalar2=mo_o[:, ct : ct + 1],
                    op0=OP.mult, op1=OP.add,
                )

          for b in range(BPC):
              x_all = x_tiles[b]
              h_all = h_tiles[b]
              q_all = qkp.tile([128, CT, N], MMDT, tag="q")
              k_all = qkp.tile([128, CT, N], MMDT, tag="k")
              vT_all = vp.tile([128, NT, C], MMDT, tag="vT")
              p_all = pp.tile([128, NT, N], MMDT, tag="p")
              recip = rp.tile([128, N], F32, tag="recip")
              hv_all = hvp.tile([128, CT, N], MMDT, tag="hv")

              # ---------------- q, k projections [o, n] ----------------
              for name, dst, bias in (("wqT", q_all, "bq"), ("wkT", k_all, "bk")):
                  for ot in range(CT):
                      for nh in range(NH):
                          ps = ps_mm.tile([128, 512], F32, tag="mm")
                          for ct in range(0, CT, KPAIR):
                              nc.tensor.matmul(
                                  ps,
                                  w_sb[name][:, ct : ct + KPAIR, ts(ot, 128)],
                                  h_all[:, ct : ct + KPAIR, ts(nh, 512)],
                                  start=(ct == 0),
                                  stop=(ct == CT - KPAIR),
                                  perf_mode=PERF,
                              )
                          nc.scalar.activation(
                              out=dst[:, ot, ts(nh, 512)], in_=ps,
                              func=AF.Identity,
                              bias=b_sb[bias][:, ot : ot + 1],
                          )

              # ---------------- vT [m, c] ----------------
              for mt in range(NT):
                  ps = ps_mm.tile([128, 512], F32, tag="mm")
                  for ct in range(0, CT, KPAIR):
                      nc.tensor.matmul(
                          ps,
                          h_all[:, ct : ct + KPAIR, ts(mt, 128)],
                          w_sb["wvT"][:, ct : ct + KPAIR, :],
                          start=(ct == 0),
                          stop=(ct == CT - KPAIR),
                          perf_mode=PERF,
                      )
                  nc.vector.tensor_copy(vT_all[:, mt, :], ps)

              # ------------- scores + exp + (lagged) denominator -------------
              psum_part = gnp.tile([128, NH, 2, 512], BF16, tag="psum_part")

              for mt in range(NT):
                  for nh in range(NH):
                      ps = ps_mm.tile([128, 512], F32, tag="mm")
                      for ot in range(0, CT, KPAIR):
                          nc.tensor.matmul(
                              ps,
                              k_all[:, ot : ot + KPAIR, ts(mt, 128)],
                              q_all[:, ot : ot + KPAIR, ts(nh, 512)],
                              start=(ot == 0),
                              stop=(ot == CT - KPAIR),
                              perf_mode=PERF,
                          )
                      nc.scalar.activation(
                          out=p_all[:, mt, ts(nh, 512)], in_=ps, func=AF.Exp,
                          scale=SCALE, bias=shift_sb,
                      )
                  if mt in (NT // 2 - 1, NT - 1):
                      hh = 0 if mt == NT // 2 - 1 else 1
                      lo = hh * (NT // 2)
                      for nh in range(NH):
                          pv = p_all[:, lo : lo + NT // 2, ts(nh, 512)]\
                              .rearrange("p m n -> p n m")
                          with nc.allow_low_precision(
                              reason="softmax denominator partials; "
                              "common-mode per column, attenuated 1e-5"
                          ):
                              nc.vector.tensor_reduce(
                                  out=psum_part[:, nh, hh, :], in_=pv,
                                  op=OP.add, axis=mybir.AxisListType.X,
                              )
              for nh in range(NH):
                  den_ps = ps_den.tile(
                      [128, 512], F32, tag="aux", name=f"den{b}_{nh}"
                  )
                  for hh in range(2):
                      nc.tensor.matmul(
                          den_ps, ones_bf, psum_part[:, nh, hh, :],
                          start=(hh == 0), stop=(hh == 1),
                      )
                  nc.vector.reciprocal(out=recip[:, ts(nh, 512)], in_=den_ps)

              # ---------------- hv = (v @ p) * recip ----------------
              for ct in range(CT):
                  for nh in range(NH):
                      ps = ps_mm.tile([128, 512], F32, tag="mm")
                      for mt in range(0, NT, KPAIR):
                          nc.tensor.matmul(
                              ps,
                              vT_all[:, mt : mt + KPAIR, ts(ct, 128)],
                              p_all[:, mt : mt + KPAIR, ts(nh, 512)],
                              start=(mt == 0),
                              stop=(mt == NT - KPAIR),
                              perf_mode=PERF,
                          )
                      nc.vector.tensor_tensor(
                          hv_all[:, ct, ts(nh, 512)], ps, recip[:, ts(nh, 512)],
                          op=OP.mult,
                      )

              # ---------------- out = woT.T @ hv + bo_eff + x ----------------
              for ot in range(CT):
                  res = resp.tile([128, N], F32, tag="res")
                  for nh in range(NH):
                      ps = ps_mm.tile([128, 512], F32, tag="mm")
                      for ct in range(0, CT, KPAIR):
                          nc.tensor.matmul(
                              ps,
                              w_sb["woT"][:, ct : ct + KPAIR, ts(ot, 128)],
                              hv_all[:, ct : ct + KPAIR, ts(nh, 512)],
                              start=(ct == 0),
                              stop=(ct == CT - KPAIR),
                              perf_mode=PERF,
                          )
                      nc.vector.scalar_tensor_tensor(
                          out=res[:, ts(nh, 512)], in0=ps,
                          scalar=b_sb["bo"][:, ot : ot + 1],
                          in1=x_all[:, ot, ts(nh, 512)],
                          op0=OP.add, op1=OP.add,
                      )
                      nc.sync.dma_start(
                          out=out_d[b, ot][:, ts(nh, 512)], in_=res[:, ts(nh, 512)]
                      )

    # The axon/PJRT path serializes nc without finalizing; Bacc's compile
    # passes (wait splitting, register allocation) must run first.
    nc.finalize()
    return nc


def _prep_inputs(x, gn_scale, gn_bias, wq, bq, wk, bk, wv, bv, wo, bo):
    bf = ml_dtypes.bfloat16
    xr = np.asarray(x, np.float32).reshape(B, CT, 128, N)
    shared = {}
    wdt = ml_dtypes.float8_e4m3 if USE_FP8 else bf
    for name, w in (("wqT", wq), ("wkT", wk), ("wvT", wv), ("woT", wo)):
        shared[name] = np.ascontiguousarray(
            np.asarray(w, np.float32).T
        ).astype(wdt).reshape(CT, 128, C)
    # bv folds into bo exactly: softmax rows sum to 1, so hv = hv_u/denom + bv
    # and wo @ (hv + bv) = wo @ hv + (wo @ bv).
    bo_eff = np.asarray(bo, np.float32) + (
        np.asarray(wo, np.float32) @ np.asarray(bv, np.float32)
    )
    vecs = [bq, bk, bo_eff, gn_scale, gn_bias]
    bvec = np.stack(
        [np.asarray(v, np.float32).reshape(CT, 128) for v in vecs]
    )  # [5, CT, 128]
    shared["bvec"] = np.ascontiguousarray(bvec.transpose(2, 0, 1).reshape(128, 5 * CT))
    gmask = np.zeros((128, GPT), np.float32)
    expand = np.zeros((GPT, 128), np.float32)
    for c in range(128):
        gmask[c, c // CPG] = 1.0 / CPG
        expand[c // CPG, c] = 1.0
    shared["gmask"] = gmask.astype(bf)
    shared["expand"] = expand.astype(bf)
    return [
        {"x": np.ascontiguousarray(xr[i * BPC : (i + 1) * BPC]), **shared}
        for i in range(NCORES)
    ]


def kernel(**inputs) -> np.ndarray:
    if "nc" not in _CACHE:
        _CACHE["nc"] = build_nc()
    in_maps = _prep_inputs(**inputs)
    res = run_bass_kernel_spmd(
        _CACHE["nc"], in_maps, core_ids=list(range(NCORES))
    )
    _CACHE["last_results"] = res
    out = np.concatenate(
        [np.asarray(r["out"], np.float32).reshape(BPC, C, N) for r in res.results],
        axis=0,
    )
    return out.reshape(B, C, H, W)

